# revision 41
# baseline (speedup 1.0000x reference)
"""DSS (Diagonal State Space) layer as a Bass/Tile kernel for 8 Trainium2 NeuronCores.

Channels H sharded 8 x 128. Per core, overlap-save FFT convolution with a
radix-8 split-DFT: each 1024-sample window is 8 chunks of 128; a short FFT8
across chunks runs on DVE in fp16 (s/d butterflies; the odd-frequency levels
are folded into merged PE stationaries), then a 128-contraction twiddle matmul
on PE produces a packed 4-plane frequency representation:
  plane A: f=8p (p<64) / f=8(p-64)+4 (p>=64), with X[0], X[512] sharing row 0
  planes 1,2,3: f = 8p + f0, p in [0,128)
(640 representative frequencies; conjugate mirrors carry weight 2 in the
inverse). Pointwise complex multiply in fp16 on DVE/GPSIMD; dense inverse DFT
back to the valid 512 samples as PE matmuls. The DSS kernel k is built
on-device: the z^(2^j) squaring chain stays fp32 (squaring doubles relative
error per level), while the GW/Z32 plane-doubling products and the mode-sum
run in fp16; the skip connection u*D folds into K (K' = K + D).
"""

import sys

for _p in ("/opt/trn_rl_repo", "/opt/trn_rl_repo/concourse"):
    if _p not in sys.path:
        sys.path.insert(0, _p)

import numpy as np
from contextlib import ExitStack

import concourse.bacc as bacc
import concourse.tile as tile
import concourse.mybir as mybir

dt = mybir.dt
f32 = np.float32

B, L, H, N = 4, 4096, 1024, 64
LK = 512
F = 1024          # FFT length (overlap-save)
HOP = 512         # block hop
NCORES = 8
HS = H // NCORES  # 128 channels per core
NBLK = L // HOP   # 8
NCH = L // 128    # 32 u chunks per core
C8 = float(np.sqrt(2.0) / 2.0)

# FS stationary index map
FS_EARP, FS_EARM, FS_EAIP, FS_EAIM = 0, 1, 2, 3
FS_C2, FS_SN2, FS_CN2 = 4, 5, 6
FS_G1RE, FS_G1IM = 7, 11          # +j, j=0..3
FS_G3RE, FS_G3IM = 15, 19
NFS = 23


# ---------------------------------------------------------------- host constants
def build_constants():
    l1 = np.arange(128)[:, None].astype(np.float64)
    p = np.arange(128)[None, :].astype(np.float64)
    FS = np.zeros((NFS, 128, 128))

    # plane A (moving t0, t1): merged E +/- tiles, Nyquist in im col 0
    fA0 = 8 * p
    fA4 = 8 * (p - 64) + 4
    m0, m4 = (p < 64), (p >= 64)
    EAr0 = np.where(m0, np.cos(2 * np.pi * l1 * fA0 / 1024), 0.0)
    EAi0 = np.where(m0 & (p > 0), -np.sin(2 * np.pi * l1 * fA0 / 1024), 0.0)
    EAi0[:, 0] = (-1.0) ** np.arange(128)
    EAr4 = np.where(m4, np.cos(2 * np.pi * l1 * fA4 / 1024), 0.0)
    EAi4 = np.where(m4, -np.sin(2 * np.pi * l1 * fA4 / 1024), 0.0)
    FS[FS_EARP], FS[FS_EARM] = EAr0 + EAr4, EAr0 - EAr4
    FS[FS_EAIP], FS[FS_EAIM] = EAi0 + EAi4, EAi0 - EAi4

    # plane 2 (moving u0, u1)
    ang2 = 2 * np.pi * l1 * (8 * p + 2) / 1024
    FS[FS_C2], FS[FS_SN2], FS[FS_CN2] = np.cos(ang2), -np.sin(ang2), -np.cos(ang2)

    # planes 1, 3 (moving d0..d3): X = sum_j exp(-i theta_j) d_j
    for f0, base_re, base_im in ((1, FS_G1RE, FS_G1IM), (3, FS_G3RE, FS_G3IM)):
        f = 8 * p + f0
        for j in range(4):
            th = 2 * np.pi * (l1 * f / 1024 + j * f0 / 8.0)
            FS[base_re + j] = np.cos(th)
            FS[base_im + j] = -np.sin(th)

    # inverse stationaries [4 planes (A,1,2,3), 4 ot, 2 (r,i), 128, 128]
    AI = np.zeros((4, 4, 2, 128, 128))
    pc = np.arange(128)[:, None].astype(np.float64)
    for ot in range(4):
        r = np.arange(128)[None, :].astype(np.float64)
        lw = 512 + 128 * ot + r
        for pi, f0 in ((1, 1), (2, 2), (3, 3)):
            f = 8 * pc + f0
            ang = 2 * np.pi * lw * f / 1024
            AI[pi, ot, 0] = 2 * np.cos(ang) / 1024
            AI[pi, ot, 1] = -2 * np.sin(ang) / 1024
        fA = np.where(pc < 64, 8 * pc, 8 * (pc - 64) + 4)
        ang = 2 * np.pi * lw * fA / 1024
        Ar = 2 * np.cos(ang) / 1024
        Ai = -2 * np.sin(ang) / 1024
        Ar[0, :] = 1.0 / 1024
        Ai[0, :] = ((-1.0) ** r[0]) / 1024
        AI[0, ot, 0], AI[0, ot, 1] = Ar, Ai
    return FS.astype(np.float16), AI.reshape(32, 128, 128).astype(np.float16)


# Horner coefficient lists (highest degree first)
def _fact(k):
    r = 1.0
    for i in range(2, k + 1):
        r *= i
    return r


EXP10 = [1.0 / _fact(k) for k in range(10, -1, -1)]
EXP9 = [1.0 / _fact(k) for k in range(9, -1, -1)]
SIN9 = [1.0 / _fact(9), -1.0 / _fact(7), 1.0 / _fact(5), -1.0 / _fact(3), 1.0]
COSC = [1.0 / _fact(10), -1.0 / _fact(8), 1.0 / _fact(6), -1.0 / _fact(4),
        1.0 / _fact(2)]


class _Prog:
    def __init__(self):
        self.nc = None
        self.built = False


_prog = _Prog()


def _emit_kernel(nc, tc, ctx, aps):
    V = nc.vector
    A = nc.scalar
    T = nc.tensor
    u_ap = aps["u"]; y_ap = aps["y"]
    TT = V.tensor_tensor
    GT = nc.gpsimd.tensor_tensor
    op = mybir.AluOpType

    # ---------------- pools
    p_fs = ctx.enter_context(tc.tile_pool(name="fs", bufs=1))
    p_ai = ctx.enter_context(tc.tile_pool(name="ai", bufs=1))
    p_uh = ctx.enter_context(tc.tile_pool(name="uh", bufs=8))
    p_sd = ctx.enter_context(tc.tile_pool(name="sd", bufs=12))
    p_xf = ctx.enter_context(tc.tile_pool(name="xf", bufs=12))
    p_yf = ctx.enter_context(tc.tile_pool(name="yf", bufs=10))
    p_tmp = ctx.enter_context(tc.tile_pool(name="tmp", bufs=4))
    p_kt = ctx.enter_context(tc.tile_pool(name="kt", bufs=1))
    p_kc = ctx.enter_context(tc.tile_pool(name="kc", bufs=4))
    p_gw = ctx.enter_context(tc.tile_pool(name="gw", bufs=1))
    p_z32 = ctx.enter_context(tc.tile_pool(name="z32", bufs=1))
    p_zp = ctx.enter_context(tc.tile_pool(name="zp", bufs=9))
    p_zp16 = ctx.enter_context(tc.tile_pool(name="zp16", bufs=4))
    p_small = ctx.enter_context(tc.tile_pool(name="small", bufs=1))
    p_gwtmp = ctx.enter_context(tc.tile_pool(name="gwtmp", bufs=1))
    p_ks = ctx.enter_context(tc.tile_pool(name="ks", bufs=4))
    p_yout = ctx.enter_context(tc.tile_pool(name="yout", bufs=4))
    p_psx = ctx.enter_context(tc.tile_pool(name="psx", bufs=4, space="PSUM"))
    p_psy = ctx.enter_context(tc.tile_pool(name="psy", bufs=2, space="PSUM"))
    p_psk = ctx.enter_context(tc.tile_pool(name="psk", bufs=2, space="PSUM"))

    # ---------------- small parameter tiles first (they gate the k prologue)
    logdt = p_small.tile([1, HS], dt.float32, tag="logdt")
    A.dma_start(logdt[:], aps["logdt"][:])
    Lre = p_small.tile([1, N], dt.float32, tag="lre")
    A.dma_start(Lre[:], aps["Lre"][:])
    Lim_r = p_small.tile([1, N], dt.float32, tag="lim")
    A.dma_start(Lim_r[:], aps["Lim"][:])
    dtile = p_small.tile([1, HS], dt.float32, tag="dtile")
    A.dma_start(dtile[:], aps["D"][:])

    # u pair tiles [128, 2, 512]: fp16 chunks (2*pr, 2*pr+1), loaded directly
    uh2 = {}

    def get_uh2(pr):
        if pr not in uh2:
            t_h = p_uh.tile([128, 2, 512], dt.float16, tag="uh", name=f"uh2_{pr}")
            for q in range(2):
                c = 2 * pr + q
                eng = nc.sync if (pr + q) % 2 == 0 else nc.scalar
                eng.dma_start(
                    t_h[:, q, :].rearrange("p (b h) -> p b h", b=4),
                    u_ap[:, 128 * c:128 * c + 128, :].transpose([1, 0, 2]))
            uh2[pr] = t_h
        return uh2[pr]

    for pr in range(4):
        get_uh2(pr)

    # ---------------- forward constant stationaries (after the first chunks)
    fs_big = p_fs.tile([128, NFS, 128], dt.float16, tag="fs")
    nc.sync.dma_start(fs_big[:, 0:12, :], aps["FS"][0:12].transpose([1, 0, 2]))
    nc.scalar.dma_start(fs_big[:, 12:NFS, :], aps["FS"][12:NFS].transpose([1, 0, 2]))

    def fs(idx):
        return fs_big[:, idx, :]

    # ---------------- forward: FFT8 butterflies (paired ops) + split-DFT matmuls
    def emit_fwd(blk):
        if blk == 0:
            s01_t, s23_t = get_uh2(0), get_uh2(1)
            d01 = p_sd.tile([128, 2, 512], dt.float16, tag="sd", name=f"d01_{blk}")
            d23 = p_sd.tile([128, 2, 512], dt.float16, tag="sd", name=f"d23_{blk}")
            V.tensor_scalar_mul(d01[:], s01_t[:], -1.0)
            V.tensor_scalar_mul(d23[:], s23_t[:], -1.0)
        else:
            s01_t = p_sd.tile([128, 2, 512], dt.float16, tag="sd", name=f"s01_{blk}")
            s23_t = p_sd.tile([128, 2, 512], dt.float16, tag="sd", name=f"s23_{blk}")
            d01 = p_sd.tile([128, 2, 512], dt.float16, tag="sd", name=f"d01_{blk}")
            d23 = p_sd.tile([128, 2, 512], dt.float16, tag="sd", name=f"d23_{blk}")
            TT(s01_t[:], get_uh2(2 * blk - 2)[:], get_uh2(2 * blk)[:], op.add)
            TT(s23_t[:], get_uh2(2 * blk - 1)[:], get_uh2(2 * blk + 1)[:], op.add)
            TT(d01[:], get_uh2(2 * blk - 2)[:], get_uh2(2 * blk)[:], op.subtract)
            TT(d23[:], get_uh2(2 * blk - 1)[:], get_uh2(2 * blk + 1)[:], op.subtract)
        t01 = p_sd.tile([128, 2, 512], dt.float16, tag="sd", name=f"t01_{blk}")
        u01 = p_sd.tile([128, 2, 512], dt.float16, tag="sd", name=f"u01_{blk}")
        TT(t01[:], s01_t[:], s23_t[:], op.add)
        TT(u01[:], s01_t[:], s23_t[:], op.subtract)
        d_ap = [d01[:, 0, :], d01[:, 1, :], d23[:, 0, :], d23[:, 1, :]]

        out = []
        for pl in range(4):
            psr = p_psx.tile([128, 512], dt.float32, tag="psx", name=f"psr{blk}_{pl}")
            psi = p_psx.tile([128, 512], dt.float32, tag="psx", name=f"psi{blk}_{pl}")
            if pl == 0:
                T.matmul(psr[:], fs(FS_EARP), t01[:, 0, :], start=True, stop=False)
                T.matmul(psr[:], fs(FS_EARM), t01[:, 1, :], start=False, stop=True)
                T.matmul(psi[:], fs(FS_EAIP), t01[:, 0, :], start=True, stop=False)
                T.matmul(psi[:], fs(FS_EAIM), t01[:, 1, :], start=False, stop=True)
            elif pl == 2:
                T.matmul(psr[:], fs(FS_C2), u01[:, 0, :], start=True, stop=False)
                T.matmul(psr[:], fs(FS_SN2), u01[:, 1, :], start=False, stop=True)
                T.matmul(psi[:], fs(FS_SN2), u01[:, 0, :], start=True, stop=False)
                T.matmul(psi[:], fs(FS_CN2), u01[:, 1, :], start=False, stop=True)
            else:
                base_re = FS_G1RE if pl == 1 else FS_G3RE
                base_im = FS_G1IM if pl == 1 else FS_G3IM
                for j in range(4):
                    T.matmul(psr[:], fs(base_re + j), d_ap[j],
                             start=(j == 0), stop=(j == 3))
                    T.matmul(psi[:], fs(base_im + j), d_ap[j],
                             start=(j == 0), stop=(j == 3))
            xp = p_xf.tile([128, 2, 512], dt.float16, tag="xf", name=f"xp{blk}_{pl}")
            A.copy(xp[:, 0, :], psr[:])
            A.copy(xp[:, 1, :], psi[:])
            out.append(xp)
        return out

    # ---------------- k prologue part 1: polynomials (fp32)
    def horner_exp(dst, x, coefs):
        pp = dst
        V.memset(pp, float(coefs[0]))
        for cc in coefs[1:]:
            tq = p_small.tile([x.shape[0], x.shape[1]], dt.float32, tag="horner", bufs=2)
            TT(tq[:], pp, x, op.mult)
            V.tensor_scalar_add(pp, tq[:], float(cc))

    x8 = p_small.tile([1, HS], dt.float32, tag="x8")
    V.tensor_scalar_mul(x8[:], logdt[:], 0.125)
    e8 = p_small.tile([1, HS], dt.float32, tag="e8")
    horner_exp(e8[:], x8[:], EXP10)
    dtv = p_small.tile([1, HS], dt.float32, tag="dtv")
    t_a = p_small.tile([1, HS], dt.float32, tag="sq1")
    TT(t_a[:], e8[:], e8[:], op.mult)
    t_b = p_small.tile([1, HS], dt.float32, tag="sq2")
    TT(t_b[:], t_a[:], t_a[:], op.mult)
    TT(dtv[:], t_b[:], t_b[:], op.mult)

    xl = p_small.tile([1, N], dt.float32, tag="xl")
    V.tensor_scalar_mul(xl[:], Lre[:], 0.125)
    el8 = p_small.tile([1, N], dt.float32, tag="el8")
    horner_exp(el8[:], xl[:], EXP10)
    t_c = p_small.tile([1, N], dt.float32, tag="sq3")
    TT(t_c[:], el8[:], el8[:], op.mult)
    t_d = p_small.tile([1, N], dt.float32, tag="sq4")
    TT(t_d[:], t_c[:], t_c[:], op.mult)
    negel = p_small.tile([1, N], dt.float32, tag="negel")
    t_e = p_small.tile([1, N], dt.float32, tag="sq5")
    TT(t_e[:], t_d[:], t_d[:], op.mult)
    V.tensor_scalar_mul(negel[:], t_e[:], -1.0)

    ps_a = p_psk.tile([128, 512], dt.float32, tag="psk")
    T.matmul(ps_a[0:N, 0:HS], negel[:], dtv[:], start=True, stop=True)
    ps_b = p_psk.tile([128, 512], dt.float32, tag="psk")
    T.matmul(ps_b[0:N, 0:HS], Lim_r[:], dtv[:], start=True, stop=True)

    ah = p_small.tile([N, HS], dt.float32, tag="ah")
    V.tensor_scalar_mul(ah[:], ps_a[0:N, 0:HS], 0.5)
    bh = p_small.tile([N, HS], dt.float32, tag="bh")
    V.tensor_scalar_mul(bh[:], ps_b[0:N, 0:HS], 0.5)
    ea = p_small.tile([N, HS], dt.float32, tag="ea")
    horner_exp(ea[:], ah[:], EXP9)
    ub = p_small.tile([N, HS], dt.float32, tag="ub")
    TT(ub[:], bh[:], bh[:], op.mult)
    sp = p_small.tile([N, HS], dt.float32, tag="sp")
    V.memset(sp[:], float(SIN9[0]))
    for cc in SIN9[1:]:
        tq = p_small.tile([N, HS], dt.float32, tag="horner", bufs=2)
        TT(tq[:], sp[:], ub[:], op.mult)
        V.tensor_scalar_add(sp[:], tq[:], float(cc))
    sb = p_small.tile([N, HS], dt.float32, tag="sb")
    TT(sb[:], sp[:], bh[:], op.mult)
    cp = p_small.tile([N, HS], dt.float32, tag="cp")
    V.memset(cp[:], float(COSC[0]))
    for cc in COSC[1:]:
        tq = p_small.tile([N, HS], dt.float32, tag="horner", bufs=2)
        TT(tq[:], cp[:], ub[:], op.mult)
        V.tensor_scalar_add(cp[:], tq[:], float(cc))
    cb = p_small.tile([N, HS], dt.float32, tag="cb")
    tq = p_small.tile([N, HS], dt.float32, tag="horner", bufs=2)
    TT(tq[:], cp[:], ub[:], op.mult)
    V.tensor_scalar(cb[:], tq[:], -1.0, 1.0, op.mult, op.add)

    wre = p_small.tile([N, HS], dt.float32, tag="wre")
    TT(wre[:], ea[:], cb[:], op.mult)
    wim = p_small.tile([N, HS], dt.float32, tag="wim")
    TT(wim[:], ea[:], sb[:], op.mult)

    # ---------------- k prologue part 2: power chains
    # squaring chain in fp32 (error doubles per level); fp16 copies feed the
    # GW/Z32 plane-doubling products which accumulate only linearly.
    def csq_parts(dre, dim_, sre, sim):
        t1_ = p_small.tile([N, HS], dt.float32, tag="csq1", bufs=2)
        TT(t1_[:], sre, sre, op.mult)
        t2_ = p_small.tile([N, HS], dt.float32, tag="csq2", bufs=2)
        TT(t2_[:], sim, sim, op.mult)
        TT(dre, t1_[:], t2_[:], op.subtract)
        t3_ = p_small.tile([N, HS], dt.float32, tag="csq3", bufs=2)
        TT(t3_[:], sre, sim, op.mult)
        V.tensor_scalar_mul(dim_, t3_[:], 2.0)

    def new_zpair(nm):
        zr = p_zp.tile([N, HS], dt.float32, tag="zp", name=f"{nm}r")
        zi = p_zp.tile([N, HS], dt.float32, tag="zp", name=f"{nm}i")
        return zr, zi

    # GW/Z32 planes stacked re (partitions 0:64) over im (64:128); each
    # doubling level is then 2 full-width mults + 2 half-width combines with
    # cross-term signs folded into the signed zim copy.
    GW_t = p_gw.tile([2 * N, HS, 32], dt.float16, tag="gw")
    GW = GW_t[:]
    wre16 = p_small.tile([N, HS], dt.float16, tag="wre16")
    A.dma_start(wre16[:], aps["Wre16"][:])
    V.tensor_scalar_mul(GW[0:N, :, 0], wre16[:], 1.0)        # re = W_re
    wim16 = p_small.tile([N, HS], dt.float16, tag="wim16")
    A.dma_start(wim16[:], aps["Wim16"][:])
    V.tensor_scalar_mul(GW[N:2 * N, :, 0], wim16[:], -1.0)   # stores -Im

    Z_t = p_z32.tile([2 * N, HS, 16], dt.float16, tag="z32")
    Z = Z_t[:]
    V.memset(Z[0:N, :, 0], 1.0)
    V.memset(Z[N:2 * N, :, 0], 0.0)

    def zstack16(zpair, sgn_top, nm):
        # returns (zre_st, zim_signed) [2N, HS] fp16:
        #   zre_st = [zre; zre],  zim_signed = [sgn_top*zim; -sgn_top*zim]
        zr = p_zp16.tile([2 * N, HS], dt.float16, tag="zp16", name=f"{nm}r16", bufs=3)
        zi = p_zp16.tile([2 * N, HS], dt.float16, tag="zp16", name=f"{nm}i16", bufs=3)
        A.copy(zr[0:N, :], zpair[0][:])
        A.copy(zr[N:2 * N, :], zpair[0][:])
        IDENT = mybir.ActivationFunctionType.Identity
        A.activation(zi[0:N, :], zpair[1][:], IDENT, scale=float(sgn_top))
        A.activation(zi[N:2 * N, :], zpair[1][:], IDENT, scale=float(-sgn_top))
        return zr, zi

    def cdouble_seg(st, zst, s0, d0, w, eng_tt):
        # st: stacked [2N, HS, nplanes]; zst = (zre_st, zim_signed)
        zre = zst[0][:].unsqueeze(2).broadcast_to([2 * N, HS, w])
        zim = zst[1][:].unsqueeze(2).broadcast_to([2 * N, HS, w])
        t42 = p_gwtmp.tile([2 * N, HS, 8], dt.float16, tag="gt2", bufs=2)
        eng_tt(t42[:, :, 0:w], st[:, :, s0:s0 + w], zim, op.mult)
        eng_tt(st[:, :, d0:d0 + w], st[:, :, s0:s0 + w], zre, op.mult)
        eng_tt(st[0:N, :, d0:d0 + w], st[0:N, :, d0:d0 + w],
               t42[N:2 * N, :, 0:w], op.add)
        eng_tt(st[N:2 * N, :, d0:d0 + w], st[N:2 * N, :, d0:d0 + w],
               t42[0:N, :, 0:w], op.add)

    # squaring chain (fp32, DVE) + GW doubling (DVE) + Z32 doubling (GPS)
    zp = []
    z0 = new_zpair("z0")
    csq_parts(z0[0][:], z0[1][:], wre[:], wim[:])
    zp.append(z0)
    zst = zstack16(z0, -1.0, "z0")
    cdouble_seg(GW, zst, 0, 1, 1, TT)
    for j in range(1, 5):
        zj = new_zpair(f"z{1 << j}")
        csq_parts(zj[0][:], zj[1][:], zp[-1][0][:], zp[-1][1][:])
        zp.append(zj)
        zst = zstack16(zj, -1.0, f"z{1 << j}")
        if j < 4:
            cdouble_seg(GW, zst, 0, 1 << j, 1 << j, TT)
        else:
            cdouble_seg(GW, zst, 0, 16, 8, TT)
            cdouble_seg(GW, zst, 8, 24, 8, TT)
    za = []
    z32t = new_zpair("z32")
    csq_parts(z32t[0][:], z32t[1][:], zp[4][0][:], zp[4][1][:])
    za.append(z32t)
    zst = zstack16(z32t, 1.0, "z32")
    cdouble_seg(Z, zst, 0, 1, 1, GT)
    for j in range(1, 4):
        zj = new_zpair(f"za{j}")
        csq_parts(zj[0][:], zj[1][:], za[-1][0][:], za[-1][1][:])
        za.append(zj)
        zst = zstack16(zj, 1.0, f"za{j}")
        cdouble_seg(Z, zst, 0, 1 << j, 1 << j, GT if j < 3 else TT)

    # forward block 0 fills engine gaps left by the serial chain
    fwd_done = {0: emit_fwd(0)}

    # inverse stationaries (needed from the first inverse, ~K-ready time)
    ai_big = []
    for pl in range(4):
        tl = p_ai.tile([128, 8, 128], dt.float16, tag=f"ai{pl}", name=f"aib{pl}")
        eng = nc.scalar if pl % 2 == 0 else nc.sync
        eng.dma_start(tl[:], aps["AI"][8 * pl:8 * pl + 8].transpose([1, 0, 2]))
        ai_big.append(tl)

    def ai(pl, ot, ri):
        return ai_big[pl][:, 2 * ot + ri, :]

    # ---------------- mode-sum: k[32a+b, h] per-channel matmuls (fp16)
    ks = []
    for g in range(4):
        kp_g = p_psk.tile([32, 32, 16], dt.float32, tag="psk", name=f"kp{g}")
        for hl in range(32):
            h = 32 * g + hl
            T.matmul(kp_g[0:32, hl, :], GW[0:N, h, :], Z[0:N, h, :],
                     start=True, stop=False)
            T.matmul(kp_g[0:32, hl, :], GW[N:2 * N, h, :], Z[N:2 * N, h, :],
                     start=False, stop=True)
        t_ks = p_ks.tile([32, 16, 32], dt.float16, tag="ks", name=f"ks{g}")
        A.copy(t_ks[:], kp_g[:].transpose([0, 2, 1]))
        ks.append(t_ks)
    kc = []
    for c in range(4):
        kc.append(p_kc.tile([128, 128], dt.float16, tag="kc", name=f"kc{c}"))
    kqi = 0
    for c in range(4):
        for g in range(4):
            for al in range(4):
                eng = nc.sync if kqi % 2 == 0 else nc.scalar
                kqi += 1
                eng.dma_start(kc[c][:][32 * al:32 * al + 32, 32 * g:32 * g + 32],
                              ks[g][0:32, 4 * c + al, :])

    fwd_done[1] = emit_fwd(1)

    # ---------------- D_rep [128, 128] (D broadcast down partitions)
    ones = p_small.tile([1, 128], dt.float32, tag="ones")
    V.memset(ones[:], 1.0)
    ps_d = p_psk.tile([128, 512], dt.float32, tag="psk")
    T.matmul(ps_d[0:128, 0:HS], ones[:], dtile[:], start=True, stop=True)
    D_rep = p_small.tile([128, 128], dt.float32, tag="drep")
    A.copy(D_rep[:], ps_d[0:128, 0:HS])

    # ---------------- K planes via the same split-DFT (k chunks, upper half 0)
    t0k = p_small.tile([128, 128], dt.float16, tag="t0k")
    t1k = p_small.tile([128, 128], dt.float16, tag="t1k")
    u0k = p_small.tile([128, 128], dt.float16, tag="u0k")
    u1k = p_small.tile([128, 128], dt.float16, tag="u1k")
    TT(t0k[:], kc[0][:], kc[2][:], op.add)
    TT(t1k[:], kc[1][:], kc[3][:], op.add)
    TT(u0k[:], kc[0][:], kc[2][:], op.subtract)
    TT(u1k[:], kc[1][:], kc[3][:], op.subtract)

    kps_re = p_psk.tile([128, 4, 128], dt.float32, tag="psk", name="kpsre")
    kps_im = p_psk.tile([128, 4, 128], dt.float32, tag="psk", name="kpsim")
    T.matmul(kps_re[:, 0, :], fs(FS_EARP), t0k[:], start=True, stop=False)
    T.matmul(kps_re[:, 0, :], fs(FS_EARM), t1k[:], start=False, stop=True)
    T.matmul(kps_im[:, 0, :], fs(FS_EAIP), t0k[:], start=True, stop=False)
    T.matmul(kps_im[:, 0, :], fs(FS_EAIM), t1k[:], start=False, stop=True)
    T.matmul(kps_re[:, 2, :], fs(FS_C2), u0k[:], start=True, stop=False)
    T.matmul(kps_re[:, 2, :], fs(FS_SN2), u1k[:], start=False, stop=True)
    T.matmul(kps_im[:, 2, :], fs(FS_SN2), u0k[:], start=True, stop=False)
    T.matmul(kps_im[:, 2, :], fs(FS_CN2), u1k[:], start=False, stop=True)
    for pl, base_re, base_im in ((1, FS_G1RE, FS_G1IM), (3, FS_G3RE, FS_G3IM)):
        for j in range(4):
            T.matmul(kps_re[:, pl, :], fs(base_re + j), kc[j][:],
                     start=(j == 0), stop=(j == 3))
            T.matmul(kps_im[:, pl, :], fs(base_im + j), kc[j][:],
                     start=(j == 0), stop=(j == 3))

    # packed variant K tensors [128, 2, 4b, 128h] fp16, D folded into the real
    # part: K1[pl] = [Kr+D | KD], K2[pl] = [Ki | Ki]; plane A: KD row0 holds
    # K[512]+D and Ki rows 0 are zeroed (the f=0/512 slots are real).
    def bc(src):
        return src.unsqueeze(1).broadcast_to([128, 4, 128])

    K1T, K2T = [], []
    for pl in range(4):
        k1 = p_kt.tile([128, 2, 4, 128], dt.float16, tag=f"k1_{pl}")
        k2 = p_kt.tile([128, 2, 4, 128], dt.float16, tag=f"k2_{pl}")
        TT(k1[:, 0], bc(kps_re[:, pl, :]), bc(D_rep[:]), op.add)
        TT(k1[:, 1], bc(kps_re[:, pl, :]), bc(D_rep[:]), op.add)
        A.copy(k2[:, 0], bc(kps_im[:, pl, :]))
        A.copy(k2[:, 1], bc(kps_im[:, pl, :]))
        K1T.append(k1)
        K2T.append(k2)
    TT(K1T[0][:, 1][0:1], bc(kps_im[:, 0, :])[0:1], bc(D_rep[:])[0:1], op.add)
    V.memset(K2T[0][:, :, :, :][0:1], 0.0)

    fwd_done[2] = emit_fwd(2)

    # ---------------- main loop
    for blk in range(NBLK):
        fwd = fwd_done.pop(blk)
        for nb in (blk + 1, blk + 2):
            if nb < NBLK and nb not in fwd_done:
                fwd_done[nb] = emit_fwd(nb)
        yr_t, yi_t = [], []
        for pl in range(4):
            xp = fwd[pl]
            k1 = K1T[pl][:].rearrange("p q b h -> p q (b h)")
            k2 = K2T[pl][:].rearrange("p q b h -> p q (b h)")
            # t14 = (Xr*Kr', Xi*KD') ; t32 = (Xr*Ki, Xi*Ki) = (t3, t2)
            t14 = p_tmp.tile([128, 2, 512], dt.float16, tag="t1")
            t32 = p_tmp.tile([128, 2, 512], dt.float16, tag="t2")
            TT(t14[:], xp[:], k1, op.mult)
            TT(t32[:], xp[:], k2, op.mult)
            yr = p_yf.tile([128, 512], dt.float16, tag="yf")
            yi = p_yf.tile([128, 512], dt.float16, tag="yf")
            if pl % 2 == 0:
                GT(yr[:], t14[:, 0, :], t32[:, 1, :], op.subtract)
                GT(yi[:], t32[:, 0, :], t14[:, 1, :], op.add)
            else:
                TT(yr[:], t14[:, 0, :], t32[:, 1, :], op.subtract)
                TT(yi[:], t32[:, 0, :], t14[:, 1, :], op.add)
            yr_t.append(yr)
            yi_t.append(yi)
        for ot in range(4):
            py = p_psy.tile([128, 512], dt.float32, tag="psy")
            for pl in range(4):
                T.matmul(py[:], ai(pl, ot, 0), yr_t[pl][:],
                         start=(pl == 0), stop=False)
                T.matmul(py[:], ai(pl, ot, 1), yi_t[pl][:],
                         start=False, stop=(pl == 3))
            c_out = 4 * blk + ot
            yo = p_yout.tile([128, 512], dt.float32, tag="yout")
            A.copy(yo[:], py[:])
            eng = nc.sync if ot % 2 == 0 else nc.scalar
            eng.dma_start(y_ap[:, 128 * c_out:128 * c_out + 128, :].transpose([1, 0, 2]),
                          yo[:].rearrange("p (b h) -> p b h", b=4))


def _build_program():
    if _prog.built:
        return
    nc = bacc.Bacc("TRN2", target_bir_lowering=False, debug=False,
                   num_devices=NCORES)
    aps = {}
    aps["u"] = nc.dram_tensor("u", [B, L, HS], dt.float16, kind="ExternalInput").ap()
    aps["D"] = nc.dram_tensor("D", [1, HS], dt.float32, kind="ExternalInput").ap()
    aps["logdt"] = nc.dram_tensor("logdt", [1, HS], dt.float32, kind="ExternalInput").ap()
    aps["Wre16"] = nc.dram_tensor("Wre16", [N, HS], dt.float16, kind="ExternalInput").ap()
    aps["Wim16"] = nc.dram_tensor("Wim16", [N, HS], dt.float16, kind="ExternalInput").ap()
    aps["Lre"] = nc.dram_tensor("Lre", [1, N], dt.float32, kind="ExternalInput").ap()
    aps["Lim"] = nc.dram_tensor("Lim", [1, N], dt.float32, kind="ExternalInput").ap()
    aps["FS"] = nc.dram_tensor("FS", [NFS, 128, 128], dt.float16,
                               kind="ExternalInput").ap()
    aps["AI"] = nc.dram_tensor("AI", [32, 128, 128], dt.float16,
                               kind="ExternalInput").ap()
    aps["y"] = nc.dram_tensor("y", [B, L, HS], dt.float32, kind="ExternalOutput").ap()
    with tile.TileContext(nc, trace_sim=False) as tc:
        with ExitStack() as ctx:
            _emit_kernel(nc, tc, ctx, aps)
    nc.compile()
    _prog.nc = nc
    _prog.FS, _prog.AI = build_constants()
    _prog.built = True


def make_in_maps(u, D, log_dt, W_re, W_im, Lambda_re, Lambda_im):
    _build_program()
    in_maps = []
    for c in range(NCORES):
        h0 = c * HS
        in_maps.append({
            "u": np.ascontiguousarray(np.asarray(u)[:, :, h0:h0 + HS],
                                      dtype=np.float16),
            "D": np.ascontiguousarray(D[h0:h0 + HS], dtype=f32).reshape(1, HS),
            "logdt": np.ascontiguousarray(log_dt[h0:h0 + HS], dtype=f32).reshape(1, HS),
            "Wre16": np.ascontiguousarray(np.asarray(W_re)[h0:h0 + HS].T,
                                          dtype=np.float16),
            "Wim16": np.ascontiguousarray(np.asarray(W_im)[h0:h0 + HS].T,
                                          dtype=np.float16),
            "Lre": np.ascontiguousarray(Lambda_re, dtype=f32).reshape(1, N),
            "Lim": np.ascontiguousarray(Lambda_im, dtype=f32).reshape(1, N),
            "FS": _prog.FS,
            "AI": _prog.AI,
        })
    return in_maps


LAST_RESULTS = None


def kernel(u, D, Lambda_re, Lambda_im, log_dt, W_re, W_im):
    global LAST_RESULTS
    from concourse.bass_utils import run_bass_kernel_spmd
    in_maps = make_in_maps(u, D, log_dt, W_re, W_im, Lambda_re, Lambda_im)
    res = run_bass_kernel_spmd(_prog.nc, in_maps, core_ids=list(range(NCORES)))
    LAST_RESULTS = res
    y = np.concatenate([res.results[c]["y"] for c in range(NCORES)], axis=2)
    return y.astype(np.float32)


# revision 71
# speedup vs baseline: 1.2233x; 1.2233x over previous
"""DSS (Diagonal State Space) layer as a Bass/Tile kernel for 8 Trainium2 NeuronCores.

Channels H sharded 8 x 128. Per core, overlap-save FFT convolution with a
radix-8 split-DFT: each 1024-sample window is 8 chunks of 128; a short FFT8
across chunks runs on DVE in fp16 (s/d butterflies; the odd-frequency levels
are folded into merged PE stationaries), then a 128-contraction twiddle matmul
on PE produces a packed 4-plane frequency representation:
  plane A: f=8p (p<64) / f=8(p-64)+4 (p>=64), with X[0], X[512] sharing row 0
  planes 1,2,3: f = 8p + f0, p in [0,128)
(640 representative frequencies; conjugate mirrors carry weight 2 in the
inverse). Pointwise complex multiply in fp16 on DVE/GPSIMD; dense inverse DFT
back to the valid 512 samples as PE matmuls. The DSS kernel k is built
on-device: the z^(2^j) squaring chain stays fp32 (squaring doubles relative
error per level), while the GW/Z32 plane-doubling products and the mode-sum
run in fp16; the skip connection u*D folds into K (K' = K + D).
"""

import os
import sys

BISECT = int(os.environ.get("KBISECT", "0"))

for _p in ("/opt/trn_rl_repo", "/opt/trn_rl_repo/concourse"):
    if _p not in sys.path:
        sys.path.insert(0, _p)

import numpy as np
from contextlib import ExitStack

import concourse.bacc as bacc
import concourse.tile as tile
import concourse.mybir as mybir

dt = mybir.dt
f32 = np.float32

B, L, H, N = 4, 4096, 1024, 64
LK = 512
F = 1024          # FFT length (overlap-save)
HOP = 512         # block hop
NCORES = 8
HS = H // NCORES  # 128 channels per core
NBLK = L // HOP   # 8
NCH = L // 128    # 32 u chunks per core
C8 = float(np.sqrt(2.0) / 2.0)

# FS stationary index map
FS_EARP, FS_EARM, FS_EAIP, FS_EAIM = 0, 1, 2, 3
FS_C2, FS_SN2, FS_CN2 = 4, 5, 6
FS_G1RE, FS_G1IM = 7, 11          # +j, j=0..3
FS_G3RE, FS_G3IM = 15, 19
NFS = 23


# ---------------------------------------------------------------- host constants
def build_constants():
    l1 = np.arange(128)[:, None].astype(np.float64)
    p = np.arange(128)[None, :].astype(np.float64)
    FS = np.zeros((NFS, 128, 128))

    # plane A (moving t0, t1): merged E +/- tiles, Nyquist in im col 0
    fA0 = 8 * p
    fA4 = 8 * (p - 64) + 4
    m0, m4 = (p < 64), (p >= 64)
    EAr0 = np.where(m0, np.cos(2 * np.pi * l1 * fA0 / 1024), 0.0)
    EAi0 = np.where(m0 & (p > 0), -np.sin(2 * np.pi * l1 * fA0 / 1024), 0.0)
    EAi0[:, 0] = (-1.0) ** np.arange(128)
    EAr4 = np.where(m4, np.cos(2 * np.pi * l1 * fA4 / 1024), 0.0)
    EAi4 = np.where(m4, -np.sin(2 * np.pi * l1 * fA4 / 1024), 0.0)
    FS[FS_EARP], FS[FS_EARM] = EAr0 + EAr4, EAr0 - EAr4
    FS[FS_EAIP], FS[FS_EAIM] = EAi0 + EAi4, EAi0 - EAi4

    # plane 2 (moving u0, u1)
    ang2 = 2 * np.pi * l1 * (8 * p + 2) / 1024
    FS[FS_C2], FS[FS_SN2], FS[FS_CN2] = np.cos(ang2), -np.sin(ang2), -np.cos(ang2)

    # planes 1, 3 (moving d0..d3): X = sum_j exp(-i theta_j) d_j
    for f0, base_re, base_im in ((1, FS_G1RE, FS_G1IM), (3, FS_G3RE, FS_G3IM)):
        f = 8 * p + f0
        for j in range(4):
            th = 2 * np.pi * (l1 * f / 1024 + j * f0 / 8.0)
            FS[base_re + j] = np.cos(th)
            FS[base_im + j] = -np.sin(th)

    # inverse stationaries [4 planes (A,1,2,3), 4 ot, 2 (r,i), 128, 128]
    AI = np.zeros((4, 4, 2, 128, 128))
    pc = np.arange(128)[:, None].astype(np.float64)
    for ot in range(4):
        r = np.arange(128)[None, :].astype(np.float64)
        lw = 512 + 128 * ot + r
        for pi, f0 in ((1, 1), (2, 2), (3, 3)):
            f = 8 * pc + f0
            ang = 2 * np.pi * lw * f / 1024
            AI[pi, ot, 0] = 2 * np.cos(ang) / 1024
            AI[pi, ot, 1] = -2 * np.sin(ang) / 1024
        fA = np.where(pc < 64, 8 * pc, 8 * (pc - 64) + 4)
        ang = 2 * np.pi * lw * fA / 1024
        Ar = 2 * np.cos(ang) / 1024
        Ai = -2 * np.sin(ang) / 1024
        Ar[0, :] = 1.0 / 1024
        Ai[0, :] = ((-1.0) ** r[0]) / 1024
        AI[0, ot, 0], AI[0, ot, 1] = Ar, Ai
    return FS.astype(np.float16), AI.reshape(32, 128, 128).astype(np.float16)


# Horner coefficient lists (highest degree first)
def _fact(k):
    r = 1.0
    for i in range(2, k + 1):
        r *= i
    return r


EXP10 = [1.0 / _fact(k) for k in range(10, -1, -1)]
EXP9 = [1.0 / _fact(k) for k in range(9, -1, -1)]
SIN9 = [1.0 / _fact(9), -1.0 / _fact(7), 1.0 / _fact(5), -1.0 / _fact(3), 1.0]
COSC = [1.0 / _fact(10), -1.0 / _fact(8), 1.0 / _fact(6), -1.0 / _fact(4),
        1.0 / _fact(2)]


class _Prog:
    def __init__(self):
        self.nc = None
        self.built = False


_prog = _Prog()


def _emit_kernel(nc, tc, ctx, aps):
    V = nc.vector
    A = nc.scalar
    T = nc.tensor
    u_ap = aps["u"]; y_ap = aps["y"]
    TT = V.tensor_tensor
    GT = nc.gpsimd.tensor_tensor
    op = mybir.AluOpType

    # ---------------- pools
    p_fs = ctx.enter_context(tc.tile_pool(name="fs", bufs=1))
    p_ai = ctx.enter_context(tc.tile_pool(name="ai", bufs=1))
    p_uh = ctx.enter_context(tc.tile_pool(name="uh", bufs=12))
    p_sd = ctx.enter_context(tc.tile_pool(name="sd", bufs=12))
    p_xf = ctx.enter_context(tc.tile_pool(name="xf", bufs=20))
    p_yf = ctx.enter_context(tc.tile_pool(name="yf", bufs=8))
    p_tmp = ctx.enter_context(tc.tile_pool(name="tmp", bufs=4))
    p_kt = ctx.enter_context(tc.tile_pool(name="kt", bufs=1))
    p_kc = ctx.enter_context(tc.tile_pool(name="kc", bufs=1))
    p_gw = ctx.enter_context(tc.tile_pool(name="gw", bufs=1))
    p_z32 = ctx.enter_context(tc.tile_pool(name="z32", bufs=1))
    p_zp = ctx.enter_context(tc.tile_pool(name="zp", bufs=6))
    p_zp16 = ctx.enter_context(tc.tile_pool(name="zp16", bufs=4))
    p_small = ctx.enter_context(tc.tile_pool(name="small", bufs=1))
    p_gwtmp = ctx.enter_context(tc.tile_pool(name="gwtmp", bufs=1))
    p_ks = ctx.enter_context(tc.tile_pool(name="ks", bufs=4))
    p_yout = ctx.enter_context(tc.tile_pool(name="yout", bufs=3))
    p_psx = ctx.enter_context(tc.tile_pool(name="psx", bufs=4, space="PSUM"))
    p_psy = ctx.enter_context(tc.tile_pool(name="psy", bufs=2, space="PSUM"))
    p_psk = ctx.enter_context(tc.tile_pool(name="psk", bufs=2, space="PSUM"))

    # ---------------- small parameter tiles first (they gate the k prologue)
    logdt = p_small.tile([1, HS], dt.float32, tag="logdt")
    A.dma_start(logdt[:], aps["logdt"][:])
    Lre = p_small.tile([1, N], dt.float32, tag="lre")
    A.dma_start(Lre[:], aps["Lre"][:])
    Lim_r = p_small.tile([1, N], dt.float32, tag="lim")
    A.dma_start(Lim_r[:], aps["Lim"][:])
    dtile = p_small.tile([1, HS], dt.float32, tag="dtile")
    A.dma_start(dtile[:], aps["D"][:])

    # u pair tiles [128, 2, 512]: fp16 chunks (2*pr, 2*pr+1), loaded directly
    uh2 = {}

    def get_uh2(pr):
        if pr not in uh2:
            t_h = p_uh.tile([128, 2, 512], dt.float16, tag="uh", name=f"uh2_{pr}")
            for q in range(2):
                c = 2 * pr + q
                eng = nc.sync if (pr + q) % 2 == 0 else nc.scalar
                eng.dma_start(
                    t_h[:, q, :].rearrange("p (b h) -> p b h", b=4),
                    u_ap[:, 128 * c:128 * c + 128, :].transpose([1, 0, 2]))
            uh2[pr] = t_h
        return uh2[pr]

    for pr in range(4):
        get_uh2(pr)

    # ---------------- forward constant stationaries (after the first chunks)
    fs_big = p_fs.tile([128, NFS, 128], dt.float16, tag="fs")
    nc.sync.dma_start(fs_big[:, 0:12, :], aps["FS"][0:12].transpose([1, 0, 2]))
    nc.scalar.dma_start(fs_big[:, 12:NFS, :], aps["FS"][12:NFS].transpose([1, 0, 2]))

    def fs(idx):
        return fs_big[:, idx, :]

    # ---------------- forward: FFT8 butterflies (paired ops) + split-DFT matmuls
    def emit_fwd(blk, FE=None):
        FE = FE or TT
        if blk == 0:
            s01_t, s23_t = get_uh2(0), get_uh2(1)
            d01 = p_sd.tile([128, 2, 512], dt.float16, tag="sd", name=f"d01_{blk}")
            d23 = p_sd.tile([128, 2, 512], dt.float16, tag="sd", name=f"d23_{blk}")
            V.tensor_scalar_mul(d01[:], s01_t[:], -1.0)
            V.tensor_scalar_mul(d23[:], s23_t[:], -1.0)
        else:
            s01_t = p_sd.tile([128, 2, 512], dt.float16, tag="sd", name=f"s01_{blk}")
            s23_t = p_sd.tile([128, 2, 512], dt.float16, tag="sd", name=f"s23_{blk}")
            d01 = p_sd.tile([128, 2, 512], dt.float16, tag="sd", name=f"d01_{blk}")
            d23 = p_sd.tile([128, 2, 512], dt.float16, tag="sd", name=f"d23_{blk}")
            FE(s01_t[:], get_uh2(2 * blk - 2)[:], get_uh2(2 * blk)[:], op.add)
            FE(s23_t[:], get_uh2(2 * blk - 1)[:], get_uh2(2 * blk + 1)[:], op.add)
            FE(d01[:], get_uh2(2 * blk - 2)[:], get_uh2(2 * blk)[:], op.subtract)
            FE(d23[:], get_uh2(2 * blk - 1)[:], get_uh2(2 * blk + 1)[:], op.subtract)
        t01 = p_sd.tile([128, 2, 512], dt.float16, tag="sd", name=f"t01_{blk}")
        u01 = p_sd.tile([128, 2, 512], dt.float16, tag="sd", name=f"u01_{blk}")
        FE(t01[:], s01_t[:], s23_t[:], op.add)
        FE(u01[:], s01_t[:], s23_t[:], op.subtract)
        d_ap = [d01[:, 0, :], d01[:, 1, :], d23[:, 0, :], d23[:, 1, :]]

        out = []
        for pl in range(4):
            psr = p_psx.tile([128, 512], dt.float32, tag="psx", name=f"psr{blk}_{pl}")
            psi = p_psx.tile([128, 512], dt.float32, tag="psx", name=f"psi{blk}_{pl}")
            if pl == 0:
                T.matmul(psr[:], fs(FS_EARP), t01[:, 0, :], start=True, stop=False)
                T.matmul(psr[:], fs(FS_EARM), t01[:, 1, :], start=False, stop=True)
                T.matmul(psi[:], fs(FS_EAIP), t01[:, 0, :], start=True, stop=False)
                T.matmul(psi[:], fs(FS_EAIM), t01[:, 1, :], start=False, stop=True)
            elif pl == 2:
                T.matmul(psr[:], fs(FS_C2), u01[:, 0, :], start=True, stop=False)
                T.matmul(psr[:], fs(FS_SN2), u01[:, 1, :], start=False, stop=True)
                T.matmul(psi[:], fs(FS_SN2), u01[:, 0, :], start=True, stop=False)
                T.matmul(psi[:], fs(FS_CN2), u01[:, 1, :], start=False, stop=True)
            else:
                base_re = FS_G1RE if pl == 1 else FS_G3RE
                base_im = FS_G1IM if pl == 1 else FS_G3IM
                for j in range(4):
                    T.matmul(psr[:], fs(base_re + j), d_ap[j],
                             start=(j == 0), stop=(j == 3))
                    T.matmul(psi[:], fs(base_im + j), d_ap[j],
                             start=(j == 0), stop=(j == 3))
            xp = p_xf.tile([128, 2, 512], dt.float16, tag="xf", name=f"xp{blk}_{pl}")
            A.copy(xp[:, 0, :], psr[:])
            A.copy(xp[:, 1, :], psi[:])
            out.append(xp)
        return out

    if BISECT == 1:
        for blk in range(NBLK):
            fwd = emit_fwd(blk)
            for ot in range(4):
                c_out = 4 * blk + ot
                yo = p_yout.tile([128, 512], dt.float32, tag="yout")
                A.copy(yo[:], fwd[ot][:].rearrange("p q f -> p (q f)")[:, 0:512])
                eng = nc.sync if ot % 2 == 0 else nc.scalar
                eng.dma_start(
                    y_ap[:, 128 * c_out:128 * c_out + 128, :].transpose([1, 0, 2]),
                    yo[:].rearrange("p (b h) -> p b h", b=4))
        return

    # ---------------- k prologue part 1: polynomials (fp32)
    def horner_exp(dst, x, coefs):
        pp = dst
        V.memset(pp, float(coefs[0]))
        for cc in coefs[1:]:
            tq = p_small.tile([x.shape[0], x.shape[1]], dt.float32, tag="horner", bufs=2)
            TT(tq[:], pp, x, op.mult)
            V.tensor_scalar_add(pp, tq[:], float(cc))

    # dt-exp and Lambda-exp share one [1, 192] Horner chain: (e^{x/8})^8
    xc = p_small.tile([1, HS + N], dt.float32, tag="xc")
    V.tensor_scalar_mul(xc[:, 0:HS], logdt[:], 0.125)
    V.tensor_scalar_mul(xc[:, HS:HS + N], Lre[:], 0.125)
    ec = p_small.tile([1, HS + N], dt.float32, tag="ec")
    horner_exp(ec[:], xc[:], EXP10)
    sq1 = p_small.tile([1, HS + N], dt.float32, tag="sq1")
    TT(sq1[:], ec[:], ec[:], op.mult)
    sq2 = p_small.tile([1, HS + N], dt.float32, tag="sq2")
    TT(sq2[:], sq1[:], sq1[:], op.mult)
    sq3 = p_small.tile([1, HS + N], dt.float32, tag="sq3")
    TT(sq3[:], sq2[:], sq2[:], op.mult)
    dtv = sq3[:, 0:HS]
    negel = p_small.tile([1, N], dt.float32, tag="negel")
    V.tensor_scalar_mul(negel[:], sq3[:, HS:HS + N], -1.0)

    ps_a = p_psk.tile([128, 512], dt.float32, tag="psk")
    T.matmul(ps_a[0:N, 0:HS], negel[:], dtv, start=True, stop=True)
    ps_b = p_psk.tile([128, 512], dt.float32, tag="psk")
    T.matmul(ps_b[0:N, 0:HS], Lim_r[:], dtv, start=True, stop=True)

    ah = p_small.tile([N, HS], dt.float32, tag="ah")
    V.tensor_scalar_mul(ah[:], ps_a[0:N, 0:HS], 0.5)
    bh = p_small.tile([N, HS], dt.float32, tag="bh")
    V.tensor_scalar_mul(bh[:], ps_b[0:N, 0:HS], 0.5)
    ea = p_small.tile([N, HS], dt.float32, tag="ea")
    horner_exp(ea[:], ah[:], EXP9)
    ub = p_small.tile([N, HS], dt.float32, tag="ub")
    TT(ub[:], bh[:], bh[:], op.mult)
    sp = p_small.tile([N, HS], dt.float32, tag="sp")
    V.memset(sp[:], float(SIN9[0]))
    for cc in SIN9[1:]:
        tq = p_small.tile([N, HS], dt.float32, tag="horner", bufs=2)
        TT(tq[:], sp[:], ub[:], op.mult)
        V.tensor_scalar_add(sp[:], tq[:], float(cc))
    sb = p_small.tile([N, HS], dt.float32, tag="sb")
    TT(sb[:], sp[:], bh[:], op.mult)
    cp = p_small.tile([N, HS], dt.float32, tag="cp")
    V.memset(cp[:], float(COSC[0]))
    for cc in COSC[1:]:
        tq = p_small.tile([N, HS], dt.float32, tag="horner", bufs=2)
        TT(tq[:], cp[:], ub[:], op.mult)
        V.tensor_scalar_add(cp[:], tq[:], float(cc))
    cb = p_small.tile([N, HS], dt.float32, tag="cb")
    tq = p_small.tile([N, HS], dt.float32, tag="horner", bufs=2)
    TT(tq[:], cp[:], ub[:], op.mult)
    V.tensor_scalar(cb[:], tq[:], -1.0, 1.0, op.mult, op.add)

    wpc = p_small.tile([N, 2, HS], dt.float32, tag="wpc")
    TT(wpc[:, 0, :], ea[:], cb[:], op.mult)
    TT(wpc[:, 1, :], ea[:], sb[:], op.mult)

    # ---------------- k prologue part 2: power chains
    # squaring chain in fp32 (error doubles per level); fp16 copies feed the
    # GW/Z32 plane-doubling products which accumulate only linearly.
    def csq_pair(dst, srcp):
        # dst, srcp: [N, 2, HS] (re, im) pair tiles; dst = srcp^2 (complex)
        t1_ = p_small.tile([N, HS], dt.float32, tag="csq1", bufs=2)
        TT(t1_[:], srcp[:, 0, :], srcp[:, 0, :], op.mult)
        t2_ = p_small.tile([N, HS], dt.float32, tag="csq2", bufs=2)
        TT(t2_[:], srcp[:, 1, :], srcp[:, 1, :], op.mult)
        TT(dst[:, 0, :], t1_[:], t2_[:], op.subtract)
        t3_ = p_small.tile([N, HS], dt.float32, tag="csq3", bufs=2)
        TT(t3_[:], srcp[:, 0, :], srcp[:, 1, :], op.mult)
        V.tensor_scalar_mul(dst[:, 1, :], t3_[:], 2.0)

    def new_zpair(nm):
        return p_zp.tile([N, 2, HS], dt.float32, tag="zp", name=nm)

    # GW/Z32 planes as separate re / (-im) tensors, plane index in the middle
    # dim so the z broadcasts keep a packed last dim (2x DVE mode). Compute
    # engines cannot shift partitions, so every op reads and writes the same
    # partition range.
    GWre_t = p_gw.tile([N, 32, HS], dt.float16, tag="gwre")
    GWim_t = p_gw.tile([N, 32, HS], dt.float16, tag="gwim")   # stores -Im
    GWre, GWim = GWre_t[:], GWim_t[:]
    wre16 = p_small.tile([N, HS], dt.float16, tag="wre16")
    A.dma_start(wre16[:], aps["Wre16"][:])
    V.tensor_scalar_mul(GWre[:, 0, :], wre16[:], 1.0)
    wim16 = p_small.tile([N, HS], dt.float16, tag="wim16")
    A.dma_start(wim16[:], aps["Wim16"][:])
    V.tensor_scalar_mul(GWim[:, 0, :], wim16[:], -1.0)

    Zre_t = p_z32.tile([N, 16, HS], dt.float16, tag="z32re")
    Zim_t = p_z32.tile([N, 16, HS], dt.float16, tag="z32im")
    Zre, Zim = Zre_t[:], Zim_t[:]
    V.memset(Zre[:, 0, :], 1.0)
    V.memset(Zim[:, 0, :], 0.0)

    def zpair16(zpair, nm):
        zc = p_zp16.tile([N, 2, HS], dt.float16, tag="zp16", name=f"{nm}16", bufs=4)
        A.copy(zc[:, 0, :], zpair[:, 0, :])
        A.copy(zc[:, 1, :], zpair[:, 1, :])
        return zc

    def cdouble_seg(pre, pim, z16, s0, d0, w, conj_stored, eng_tt, eng_cross=None):
        zre = z16[:, 0, :].unsqueeze(1).broadcast_to([N, w, HS])
        zim = z16[:, 1, :].unsqueeze(1).broadcast_to([N, w, HS])
        t2_ = p_gwtmp.tile([N, 8, HS], dt.float16, tag="gt2", bufs=2)
        t4_ = p_gwtmp.tile([N, 8, HS], dt.float16, tag="gt2", bufs=2)
        XC = eng_cross or eng_tt
        XC(t2_[:, 0:w, :], pim[:, s0:s0 + w, :], zim, op.mult)
        XC(t4_[:, 0:w, :], pre[:, s0:s0 + w, :], zim, op.mult)
        eng_tt(pre[:, d0:d0 + w, :], pre[:, s0:s0 + w, :], zre, op.mult)
        eng_tt(pim[:, d0:d0 + w, :], pim[:, s0:s0 + w, :], zre, op.mult)
        eng_tt(pre[:, d0:d0 + w, :], pre[:, d0:d0 + w, :], t2_[:, 0:w, :],
               op.add if conj_stored else op.subtract)
        eng_tt(pim[:, d0:d0 + w, :], pim[:, d0:d0 + w, :], t4_[:, 0:w, :],
               op.subtract if conj_stored else op.add)

    zp = []
    z0 = new_zpair("z0")
    csq_pair(z0, wpc)
    zp.append(z0)
    z16 = zpair16(z0, "z0")
    cdouble_seg(GWre, GWim, z16, 0, 1, 1, True, TT)
    for j in range(1, 5):
        zj = new_zpair(f"z{1 << j}")
        csq_pair(zj, zp[-1])
        zp.append(zj)
        z16 = zpair16(zj, f"z{1 << j}")
        if j < 4:
            cdouble_seg(GWre, GWim, z16, 0, 1 << j, 1 << j, True, TT)
        else:
            cdouble_seg(GWre, GWim, z16, 0, 16, 8, True, TT)
            cdouble_seg(GWre, GWim, z16, 8, 24, 8, True, TT)
    za = []
    z32t = new_zpair("z32")
    csq_pair(z32t, zp[4])
    za.append(z32t)
    z16 = zpair16(z32t, "z32")
    cdouble_seg(Zre, Zim, z16, 0, 1, 1, False, GT)
    for j in range(1, 4):
        zj = new_zpair(f"za{j}")
        csq_pair(zj, za[-1])
        za.append(zj)
        z16 = zpair16(zj, f"za{j}")
        cdouble_seg(Zre, Zim, z16, 0, 1 << j, 1 << j, False, GT if j < 2 else TT)

    # forward block 0 fills engine gaps left by the serial chain; blocks 3-4
    # run their butterflies on GPSIMD so PE has forward work during the chain
    fwd_done = {0: emit_fwd(0)}
    fwd_done[3] = emit_fwd(3, FE=GT)
    fwd_done[4] = emit_fwd(4, FE=GT)

    # inverse stationaries (needed from the first inverse, ~K-ready time)
    ai_big = []
    for pl in range(4):
        tl = p_ai.tile([128, 8, 128], dt.float16, tag=f"ai{pl}", name=f"aib{pl}")
        eng = nc.scalar if pl % 2 == 0 else nc.sync
        eng.dma_start(tl[:], aps["AI"][8 * pl:8 * pl + 8].transpose([1, 0, 2]))
        ai_big.append(tl)

    def ai(pl, ot, ri):
        return ai_big[pl][:, 2 * ot + ri, :]

    # ---------------- mode-sum: k[32a+b, h] per-channel matmuls (fp16)
    ks = []
    for g in range(4):
        kp_g = p_psk.tile([32, 32, 16], dt.float32, tag="psk", name=f"kp{g}")
        for hl in range(32):
            h = 32 * g + hl
            T.matmul(kp_g[0:32, hl, :], GWre[:, :, h], Zre[:, :, h],
                     start=True, stop=False)
            T.matmul(kp_g[0:32, hl, :], GWim[:, :, h], Zim[:, :, h],
                     start=False, stop=True)
        t_ks = p_ks.tile([32, 16, 32], dt.float16, tag="ks", name=f"ks{g}")
        if g % 2 == 0:
            A.copy(t_ks[:], kp_g[:].transpose([0, 2, 1]))
        else:
            V.tensor_scalar_mul(t_ks[:], kp_g[:].transpose([0, 2, 1]), 1.0)
        ks.append(t_ks)
    # one [128 l1, 4 c, 128 h] tensor; per (g, al): partitions 32al+b get
    # a-indices al::4 from ks[g] in one strided 3-dim DMA (16 total).
    kcall = p_kc.tile([128, 4, 128], dt.float16, tag="kc", name="kcall")
    kqi = 0
    for g in range(4):
        for al in range(4):
            eng = nc.sync if kqi % 2 == 0 else nc.scalar
            kqi += 1
            eng.dma_start(kcall[32 * al:32 * al + 32, :, 32 * g:32 * g + 32],
                          ks[g][0:32, al:16:4, :])
    kc = [kcall[:, c, :] for c in range(4)]

    fwd_done[1] = emit_fwd(1)

    # ---------------- K planes via the same split-DFT (k chunks, upper half 0)
    # the skip connection folds in as k[0] += D (K'[f] = K[f] + D for all f)
    TT(kc[0][0:1, :], kc[0][0:1, :], dtile[:], op.add)
    t0k = p_small.tile([128, 128], dt.float16, tag="t0k")
    t1k = p_small.tile([128, 128], dt.float16, tag="t1k")
    u0k = p_small.tile([128, 128], dt.float16, tag="u0k")
    u1k = p_small.tile([128, 128], dt.float16, tag="u1k")
    TT(t0k[:], kc[0], kc[2], op.add)
    TT(t1k[:], kc[1], kc[3], op.add)
    TT(u0k[:], kc[0], kc[2], op.subtract)
    TT(u1k[:], kc[1], kc[3], op.subtract)

    kps_re = p_psk.tile([128, 4, 128], dt.float32, tag="psk", name="kpsre")
    kps_im = p_psk.tile([128, 4, 128], dt.float32, tag="psk", name="kpsim")
    T.matmul(kps_re[:, 0, :], fs(FS_EARP), t0k[:], start=True, stop=False)
    T.matmul(kps_re[:, 0, :], fs(FS_EARM), t1k[:], start=False, stop=True)
    T.matmul(kps_im[:, 0, :], fs(FS_EAIP), t0k[:], start=True, stop=False)
    T.matmul(kps_im[:, 0, :], fs(FS_EAIM), t1k[:], start=False, stop=True)
    T.matmul(kps_re[:, 2, :], fs(FS_C2), u0k[:], start=True, stop=False)
    T.matmul(kps_re[:, 2, :], fs(FS_SN2), u1k[:], start=False, stop=True)
    T.matmul(kps_im[:, 2, :], fs(FS_SN2), u0k[:], start=True, stop=False)
    T.matmul(kps_im[:, 2, :], fs(FS_CN2), u1k[:], start=False, stop=True)
    for pl, base_re, base_im in ((1, FS_G1RE, FS_G1IM), (3, FS_G3RE, FS_G3IM)):
        for j in range(4):
            T.matmul(kps_re[:, pl, :], fs(base_re + j), kc[j][:],
                     start=(j == 0), stop=(j == 3))
            T.matmul(kps_im[:, pl, :], fs(base_im + j), kc[j][:],
                     start=(j == 0), stop=(j == 3))

    # packed variant K tensors [128, 2, 4b, 128h] fp16, D folded into the real
    # part: K1[pl] = [Kr+D | KD], K2[pl] = [Ki | Ki]; plane A: KD row0 holds
    # K[512]+D and Ki rows 0 are zeroed (the f=0/512 slots are real).
    def bc(src):
        return src.unsqueeze(1).broadcast_to([128, 4, 128])

    K1T, K2T = [], []
    GTS = nc.gpsimd.tensor_scalar_mul
    for pl in range(4):
        k1 = p_kt.tile([128, 2, 4, 128], dt.float16, tag=f"k1_{pl}")
        k2 = p_kt.tile([128, 2, 4, 128], dt.float16, tag=f"k2_{pl}")
        A.copy(k1[:, 0], bc(kps_re[:, pl, :]))
        GTS(k1[:, 1], bc(kps_re[:, pl, :]), 1.0)
        A.copy(k2[:, 0], bc(kps_im[:, pl, :]))
        GTS(k2[:, 1], bc(kps_im[:, pl, :]), 1.0)
        K1T.append(k1)
        K2T.append(k2)
    A.copy(K1T[0][:, 1][0:1], bc(kps_im[:, 0, :])[0:1])
    V.memset(K2T[0][:, :, :, :][0:1], 0.0)

    fwd_done[2] = emit_fwd(2)

    if BISECT == 2:
        for blk in range(NBLK):
            fwd = fwd_done.pop(blk)
            for nb in (blk + 1, blk + 2):
                if nb < NBLK and nb not in fwd_done:
                    fwd_done[nb] = emit_fwd(nb)
            for ot in range(4):
                c_out = 4 * blk + ot
                yo = p_yout.tile([128, 512], dt.float32, tag="yout")
                A.copy(yo[:], fwd[ot][:].rearrange("p q f -> p (q f)")[:, 0:512])
                eng = nc.sync if ot % 2 == 0 else nc.scalar
                eng.dma_start(
                    y_ap[:, 128 * c_out:128 * c_out + 128, :].transpose([1, 0, 2]),
                    yo[:].rearrange("p (b h) -> p b h", b=4))
        return

    # ---------------- main loop
    for blk in range(NBLK):
        fwd = fwd_done.pop(blk)
        for nb in (blk + 1, blk + 2):
            if nb < NBLK and nb not in fwd_done:
                fwd_done[nb] = emit_fwd(nb)
        yr_t, yi_t = [], []
        for pl in range(4):
            xp = fwd[pl]
            k1 = K1T[pl][:].rearrange("p q b h -> p q (b h)")
            k2 = K2T[pl][:].rearrange("p q b h -> p q (b h)")
            # t14 = (Xr*Kr', Xi*KD') ; t32 = (Xr*Ki, Xi*Ki) = (t3, t2)
            t14 = p_tmp.tile([128, 2, 512], dt.float16, tag="t1")
            t32 = p_tmp.tile([128, 2, 512], dt.float16, tag="t2")
            TT(t14[:], xp[:], k1, op.mult)
            TT(t32[:], xp[:], k2, op.mult)
            yr = p_yf.tile([128, 512], dt.float16, tag="yf")
            yi = p_yf.tile([128, 512], dt.float16, tag="yf")
            PT = GT if (pl % 2 == 0 and blk >= 4) else TT
            PT(yr[:], t14[:, 0, :], t32[:, 1, :], op.subtract)
            PT(yi[:], t32[:, 0, :], t14[:, 1, :], op.add)
            yr_t.append(yr)
            yi_t.append(yi)
        for ot in range(4):
            py = p_psy.tile([128, 512], dt.float32, tag="psy")
            for pl in range(4):
                T.matmul(py[:], ai(pl, ot, 0), yr_t[pl][:],
                         start=(pl == 0), stop=False)
                T.matmul(py[:], ai(pl, ot, 1), yi_t[pl][:],
                         start=False, stop=(pl == 3))
            c_out = 4 * blk + ot
            yo = p_yout.tile([128, 512], dt.float32, tag="yout")
            A.copy(yo[:], py[:])
            eng = nc.sync if ot % 2 == 0 else nc.scalar
            eng.dma_start(y_ap[:, 128 * c_out:128 * c_out + 128, :].transpose([1, 0, 2]),
                          yo[:].rearrange("p (b h) -> p b h", b=4))


def _build_program():
    if _prog.built:
        return
    nc = bacc.Bacc("TRN2", target_bir_lowering=False, debug=False,
                   num_devices=NCORES)
    aps = {}
    aps["u"] = nc.dram_tensor("u", [B, L, HS], dt.float16, kind="ExternalInput").ap()
    aps["D"] = nc.dram_tensor("D", [1, HS], dt.float32, kind="ExternalInput").ap()
    aps["logdt"] = nc.dram_tensor("logdt", [1, HS], dt.float32, kind="ExternalInput").ap()
    aps["Wre16"] = nc.dram_tensor("Wre16", [N, HS], dt.float16, kind="ExternalInput").ap()
    aps["Wim16"] = nc.dram_tensor("Wim16", [N, HS], dt.float16, kind="ExternalInput").ap()
    aps["Lre"] = nc.dram_tensor("Lre", [1, N], dt.float32, kind="ExternalInput").ap()
    aps["Lim"] = nc.dram_tensor("Lim", [1, N], dt.float32, kind="ExternalInput").ap()
    aps["FS"] = nc.dram_tensor("FS", [NFS, 128, 128], dt.float16,
                               kind="ExternalInput").ap()
    aps["AI"] = nc.dram_tensor("AI", [32, 128, 128], dt.float16,
                               kind="ExternalInput").ap()
    aps["y"] = nc.dram_tensor("y", [B, L, HS], dt.float32, kind="ExternalOutput").ap()
    with tile.TileContext(nc, trace_sim=False) as tc:
        with ExitStack() as ctx:
            _emit_kernel(nc, tc, ctx, aps)
    nc.compile()
    _prog.nc = nc
    _prog.FS, _prog.AI = build_constants()
    _prog.built = True


def make_in_maps(u, D, log_dt, W_re, W_im, Lambda_re, Lambda_im):
    _build_program()
    in_maps = []
    for c in range(NCORES):
        h0 = c * HS
        in_maps.append({
            "u": np.ascontiguousarray(np.asarray(u)[:, :, h0:h0 + HS],
                                      dtype=np.float16),
            "D": np.ascontiguousarray(D[h0:h0 + HS], dtype=f32).reshape(1, HS),
            "logdt": np.ascontiguousarray(log_dt[h0:h0 + HS], dtype=f32).reshape(1, HS),
            "Wre16": np.ascontiguousarray(np.asarray(W_re)[h0:h0 + HS].T,
                                          dtype=np.float16),
            "Wim16": np.ascontiguousarray(np.asarray(W_im)[h0:h0 + HS].T,
                                          dtype=np.float16),
            "Lre": np.ascontiguousarray(Lambda_re, dtype=f32).reshape(1, N),
            "Lim": np.ascontiguousarray(Lambda_im, dtype=f32).reshape(1, N),
            "FS": _prog.FS,
            "AI": _prog.AI,
        })
    return in_maps


LAST_RESULTS = None


def kernel(u, D, Lambda_re, Lambda_im, log_dt, W_re, W_im):
    global LAST_RESULTS
    from concourse.bass_utils import run_bass_kernel_spmd
    in_maps = make_in_maps(u, D, log_dt, W_re, W_im, Lambda_re, Lambda_im)
    res = run_bass_kernel_spmd(_prog.nc, in_maps, core_ids=list(range(NCORES)))
    LAST_RESULTS = res
    y = np.concatenate([res.results[c]["y"] for c in range(NCORES)], axis=2)
    return y.astype(np.float32)
